# revision 29
# baseline (speedup 1.0000x reference)
"""ChebConv (K=3) Trainium2 kernel, 8-core SPMD.

Math: with lam = lambda_max, c1=-2/lam, c2=2/lam-1, d1=-4/lam, d2=4/lam-2 and
A = D^-1/2 A D^-1/2 (in-degree norm, clamped), the reference output is

    out = feat @ M0 + g @ M1 + q @ M2 + bias,   g = A feat, q = A g
    M0 = W0^T + c2 W1^T + (d2 c2 - 1) W2^T
    M1 = c1 W1^T + (d1 c2 + d2 c1) W2^T
    M2 = d1 c1 W2^T

Device strategy (one NEFF, SPMD on 8 cores):
  - dst nodes padded to a multiple of 8*128 and block-partitioned (128/block),
    98 blocks per core. Edges are bucketed by (dst block, src chunk) on host,
    sorted by src within each bucket (HBM row locality for the gathers);
    each bucket is padded to a multiple of 128 "edge tiles".
  - inputs are minimized for host->device transfer: only the local feature
    shard is uploaded (fp16); the full feature table is assembled on-device
    with an AllGather. Gather indices are uploaded in compact [16, n/16]
    form and replicated 8x across partitions in DRAM on-device. Per-edge
    weights/dst-lane arrays are fp16. Output is fp16 (cast to fp32 on host).
  - per edge tile: dma_gather 128 source rows (fp16, 256B each) from HBM;
    build a weighted one-hot [128e x 128dst] via one fused tensor_scalar
    (iota == dl) * w with w = norm[src]*norm[dst] (0 for padding); matmul
    lhsT=X_tile rhs=onehot accumulating g^T block [128f x 128dst] in PSUM.
  - g blocks are transposed back to node-major via an identity matmul and
    written to a DRAM bounce buffer; one fp16 AllGather shares g across
    cores; hop 2 repeats the same structure on g to get q.
  - dense epilogue per block on TensorE with host-folded M0/M1/M2 + bias.
"""
import os
import sys

sys.path.insert(0, "/opt/trn_rl_repo")

import numpy as np

import concourse.bacc as bacc
import concourse.mybir as mybir
import concourse.tile as tile
from concourse import bass_utils

NCORE = 8
BLK = 128
D = 128
NCHUNK = 4
CALL_TILES = 32                      # edge tiles per dma_gather call
CALL_IDX = CALL_TILES * BLK


def _prep(feat, W, bias, lambda_max, src, dst):
    """Host-side graph preprocessing. Returns per-core in_maps + plan."""
    N = feat.shape[0]
    E = src.shape[0]
    src = np.asarray(src).astype(np.int64)
    dst = np.asarray(dst).astype(np.int64)
    feat = np.asarray(feat).astype(np.float32)
    W = np.asarray(W).astype(np.float32)
    bias = np.asarray(bias).astype(np.float32)
    lam = float(np.asarray(lambda_max).reshape(-1)[0])

    npad_unit = NCORE * BLK
    NPAD = ((N + npad_unit - 1) // npad_unit) * npad_unit
    NBLK = NPAD // BLK
    BPC = NBLK // NCORE
    NPC = BPC * BLK
    CHUNK = NPAD // NCHUNK
    assert CHUNK % BLK == 0 and CHUNK < 32767, (NPAD, CHUNK)

    # normalization
    deg = np.bincount(dst, minlength=N).astype(np.float32)
    norm = np.clip(deg, 1.0, None) ** -0.5
    w_all = (norm[src] * norm[dst]).astype(np.float32)

    blk_all = dst // BLK                      # global dst block
    chunk_all = src // CHUNK
    key = (blk_all * NCHUNK + chunk_all).astype(np.int64)
    # sort by bucket, then by src within bucket: the gather's HBM reads
    # then walk ascending addresses (row locality) instead of random.
    order = np.argsort(key * (1 << 17) + src, kind="stable")
    sk = key[order]

    cnt_flat = np.bincount(key, minlength=NBLK * NCHUNK)
    cnt = cnt_flat.reshape(NCORE, BPC, NCHUNK)
    # tiles per (block-within-core, chunk): max over cores -> shared program
    T = -(-cnt.max(axis=0) // BLK)            # [BPC, NCHUNK]
    # every block needs at least one tile so its PSUM group gets start/stop
    none_mask = T.sum(axis=1) == 0
    T[none_mask, 0] = 1
    tile_off = np.zeros((BPC, NCHUNK), np.int64)
    NT = np.zeros(NCHUNK, np.int64)
    for c in range(NCHUNK):
        tile_off[:, c] = np.cumsum(T[:, c]) - T[:, c]
        NT[c] = T[:, c].sum()

    # slot position of every edge inside its core's per-chunk stream
    group_starts = np.zeros(NBLK * NCHUNK + 1, np.int64)
    group_starts[1:] = np.cumsum(cnt_flat)
    rank = np.arange(E, dtype=np.int64) - group_starts[sk]
    bb_s = (sk // NCHUNK) % BPC
    core_s = (sk // NCHUNK) // BPC
    c_s = sk % NCHUNK
    pos = tile_off[bb_s, c_s] * BLK + rank

    idx16_all = (src - chunk_all * CHUNK).astype(np.int16)[order]
    w_s = w_all[order].astype(np.float16)
    dl_s = (dst % BLK).astype(np.float16)[order]

    idxs = [np.zeros((NCORE, NT[c] * BLK), np.int16) for c in range(NCHUNK)]
    ws = [np.zeros((NCORE, NT[c] * BLK), np.float16) for c in range(NCHUNK)]
    dls = [np.zeros((NCORE, NT[c] * BLK), np.float16) for c in range(NCHUNK)]
    for c in range(NCHUNK):
        m = c_s == c
        idxs[c][core_s[m], pos[m]] = idx16_all[m]
        ws[c][core_s[m], pos[m]] = w_s[m]
        dls[c][core_s[m], pos[m]] = dl_s[m]

    # folded dense matrices
    c1 = -2.0 / lam
    c2 = 2.0 / lam - 1.0
    d1 = -4.0 / lam
    d2 = 4.0 / lam - 2.0
    W0T, W1T, W2T = W[0].T, W[1].T, W[2].T
    M0 = W0T + c2 * W1T + (d2 * c2 - 1.0) * W2T
    M1 = c1 * W1T + (d1 * c2 + d2 * c1) * W2T
    M2 = (d1 * c1) * W2T

    featH = np.zeros((NPAD, D), np.float16)
    featH[:N] = feat.astype(np.float16)

    shared = {
        "M0": M0.astype(np.float16),
        "M1": M1.astype(np.float16),
        "M2": M2.astype(np.float16),
        "bias_rep": np.tile(bias[None, :].astype(np.float32), (BLK, 1)),
        "iota": np.tile(np.arange(BLK, dtype=np.float16)[None, :], (BLK, 1)),
        "ident": np.eye(BLK, dtype=np.float16),
    }
    in_maps = []
    for k in range(NCORE):
        m = dict(shared)
        m["featLocal"] = featH[k * NPC : (k + 1) * NPC]
        for c in range(NCHUNK):
            m[f"idx{c}"] = np.ascontiguousarray(idxs[c][k].reshape(-1, 16).T)
            m[f"w{c}"] = np.ascontiguousarray(ws[c][k].reshape(-1, BLK).T)
            m[f"dl{c}"] = np.ascontiguousarray(dls[c][k].reshape(-1, BLK).T)
        in_maps.append(m)

    plan = dict(N=N, NPAD=NPAD, BPC=BPC, NPC=NPC, CHUNK=CHUNK,
                T=T, tile_off=tile_off, NT=NT)
    return in_maps, plan


def _build(plan, variant="full", reps=1, act_every=0, sbufs=3):
    """Emit the Bass/Tile program for the shared SPMD NEFF.

    variant="full": the real kernel (feat AllGather -> hop1 -> AllGather ->
    hop2+epilogue).
    variant="timing_hops"/"timing_gather"/"timing_compute"/"timing_nodep":
    no collectives; hops wrapped in a For_i(reps) hardware loop for
    differential wall-clock timing. timing_compute runs the compute with a
    constant lhsT (no gathers); timing_nodep runs gathers AND compute but
    with no data dependency between them (overlap ceiling).
    """
    BPC, NPC, NPAD, CHUNK = plan["BPC"], plan["NPC"], plan["NPAD"], plan["CHUNK"]
    T, tile_off, NT = plan["T"], plan["tile_off"], plan["NT"]
    f16, f32, i16 = mybir.dt.float16, mybir.dt.float32, mybir.dt.int16

    nc = bacc.Bacc("TRN2", target_bir_lowering=False, debug=False,
                   num_devices=NCORE, num_swdge_queues=4)
    featL_d = nc.dram_tensor("featLocal", [NPC, D], f16, kind="ExternalInput")
    idx_d = [nc.dram_tensor(f"idx{c}", [16, NT[c] * 8], i16, kind="ExternalInput")
             for c in range(NCHUNK)]
    w_d = [nc.dram_tensor(f"w{c}", [128, NT[c]], f16, kind="ExternalInput")
           for c in range(NCHUNK)]
    dl_d = [nc.dram_tensor(f"dl{c}", [128, NT[c]], f16, kind="ExternalInput")
            for c in range(NCHUNK)]
    M_d = [nc.dram_tensor(f"M{i}", [D, D], f16, kind="ExternalInput")
           for i in range(3)]
    bias_d = nc.dram_tensor("bias_rep", [BLK, D], f32, kind="ExternalInput")
    iota_d = nc.dram_tensor("iota", [BLK, BLK], f16, kind="ExternalInput")
    ident_d = nc.dram_tensor("ident", [BLK, BLK], f16, kind="ExternalInput")
    out_d = nc.dram_tensor("out", [NPC, D], f16, kind="ExternalOutput")

    with tile.TileContext(nc) as tc:
        with (
            tc.tile_pool(name="const", bufs=1) as cpool,
            tc.tile_pool(name="resident", bufs=1) as rpool,
            tc.tile_pool(name="idxp", bufs=6) as idxpool,
            tc.tile_pool(name="streams", bufs=sbufs) as spool,
            tc.tile_pool(name="ow", bufs=8) as owpool,
            tc.tile_pool(name="small", bufs=3) as npool,
            tc.tile_pool(name="psum", bufs=2, space="PSUM") as psum,
            tc.tile_pool(name="dram", bufs=1, space="DRAM") as dram,
        ):
            iota_t = cpool.tile([BLK, BLK], f16)
            nc.sync.dma_start(out=iota_t[:], in_=iota_d[:])
            ident_t = cpool.tile([BLK, BLK], f16)
            nc.sync.dma_start(out=ident_t[:], in_=ident_d[:])
            M_t = []
            for i in range(3):
                t = cpool.tile([D, D], f16, tag=f"M{i}")
                nc.sync.dma_start(out=t[:], in_=M_d[i][:])
                M_t.append(t)
            bias_t = cpool.tile([BLK, D], f32)
            nc.sync.dma_start(out=bias_t[:], in_=bias_d[:])
            # w/dl ship as f16 and are widened on-device: tensor_scalar /
            # activation AP scalar operands must be f32. negw feeds the ACT
            # one-hot path's scale operand.
            w_t, dl_t, negw_t = [], [], []
            with tc.tile_pool(name="stage16", bufs=2) as stpool:
                for c in range(NCHUNK):
                    st_w = stpool.tile([128, NT[c]], f16, tag="st")
                    nc.sync.dma_start(out=st_w[:], in_=w_d[c][:])
                    wt = rpool.tile([128, NT[c]], f32, tag=f"w{c}")
                    nc.vector.tensor_copy(out=wt[:], in_=st_w[:])
                    w_t.append(wt)
                    nwt = rpool.tile([128, NT[c]], f32, tag=f"negw{c}")
                    nc.vector.tensor_scalar(out=nwt[:], in0=st_w[:],
                                            scalar1=-1.0, scalar2=None,
                                            op0=mybir.AluOpType.mult)
                    negw_t.append(nwt)
                    st_d = stpool.tile([128, NT[c]], f16, tag="st")
                    nc.sync.dma_start(out=st_d[:], in_=dl_d[c][:])
                    dt_ = rpool.tile([128, NT[c]], f32, tag=f"dl{c}")
                    nc.vector.tensor_copy(out=dt_[:], in_=st_d[:])
                    dl_t.append(dt_)
            featT = rpool.tile([128, NPC], f16, tag="featT")
            nc.sync.dma_start_transpose(out=featT[:], in_=featL_d[:])
            gT = rpool.tile([128, NPC], f16, tag="gT")

            # negated iota for the ACT-engine one-hot path:
            # t = |(-iota) + dl|, ow = relu(t * (-w) + w) = w * (iota == dl).
            negiota_t = cpool.tile([BLK, BLK], f16)
            nc.vector.tensor_scalar(out=negiota_t[:], in0=iota_t[:],
                                    scalar1=-1.0, scalar2=None,
                                    op0=mybir.AluOpType.mult)

            # on-device replication of the compact index arrays: 8 copies
            # of [16, n] stacked into the [128, n] layout dma_gather reads.
            idx_full = []
            for c in range(NCHUNK):
                t = dram.tile([128, NT[c] * 8], i16, tag=f"idxfull{c}")
                for k in range(8):
                    nc.sync.dma_start(out=t[16 * k : 16 * (k + 1), :],
                                      in_=idx_d[c][:])
                idx_full.append(t)

            featH = dram.tile([NPAD, D], f16)
            featB = dram.tile([NPC, D], f16)
            cc_in = dram.tile([NPC, D], f16)
            cc_out = dram.tile([NPAD, D], f16)

            def run_hop(src_views, out_hook, skip_gather=False,
                        skip_compute=False, nodep=False, noow=False):
                emitted = [0] * NCHUNK
                bufs = {}

                def ensure_call(c, j):
                    while emitted[c] <= j:
                        jj = emitted[c]
                        n_t = min(CALL_TILES, int(NT[c]) - jj * CALL_TILES)
                        n_idx = n_t * BLK
                        ib = idxpool.tile([128, CALL_IDX // 16], i16, tag="idx")
                        nc.sync.dma_start(
                            out=ib[:, : n_idx // 16],
                            in_=idx_full[c][:, jj * (CALL_IDX // 16):
                                            jj * (CALL_IDX // 16) + n_idx // 16],
                        )
                        buf = spool.tile([128, CALL_TILES, BLK], f16, tag=f"s{c}")
                        nc.gpsimd.dma_gather(
                            out_ap=buf[:, :n_t, :],
                            in_ap=src_views[c],
                            idxs_ap=ib[:, : n_idx // 16],
                            num_idxs=n_idx,
                            num_idxs_reg=n_idx,
                            elem_size=D,
                            single_packet=False,
                            queue_num=c,
                        )
                        bufs[(c, jj)] = buf
                        emitted[c] += 1

                for bb in range(BPC):
                    total = int(T[bb].sum())
                    if not skip_compute:
                        acc = psum.tile([128, BLK], f32, tag="acc", space="PSUM")
                    done = 0
                    for c in range(NCHUNK):
                        for t in range(int(T[bb][c])):
                            p = int(tile_off[bb][c]) + t
                            j, slot = divmod(p, CALL_TILES)
                            if not skip_gather:
                                ensure_call(c, j)
                            if skip_compute:
                                continue
                            if noow:
                                done += 1
                                nc.tensor.matmul(
                                    out=acc[:],
                                    lhsT=bufs[(c, j)][:, slot, :],
                                    rhs=iota_t[:],
                                    start=(done == 1),
                                    stop=(done == total),
                                )
                                continue
                            ow = owpool.tile([128, BLK], f16, tag="ow")
                            if act_every and done % act_every == 1:
                                # ACT-engine one-hot (own SBUF ports; keeps
                                # DVE short so SWDGE descgen is not starved)
                                at = owpool.tile([128, BLK], f16, tag="act_t")
                                nc.scalar.activation(
                                    out=at[:], in_=negiota_t[:],
                                    func=mybir.ActivationFunctionType.Abs,
                                    bias=dl_t[c][:, p : p + 1],
                                )
                                nc.scalar.activation(
                                    out=ow[:], in_=at[:],
                                    func=mybir.ActivationFunctionType.Relu,
                                    bias=w_t[c][:, p : p + 1],
                                    scale=negw_t[c][:, p : p + 1],
                                )
                            else:
                                nc.vector.tensor_scalar(
                                    out=ow[:],
                                    in0=iota_t[:],
                                    scalar1=dl_t[c][:, p : p + 1],
                                    scalar2=w_t[c][:, p : p + 1],
                                    op0=mybir.AluOpType.is_equal,
                                    op1=mybir.AluOpType.mult,
                                )
                            done += 1
                            lhsT = (iota_t[:] if (skip_gather or nodep)
                                    else bufs[(c, j)][:, slot, :])
                            nc.tensor.matmul(
                                out=acc[:],
                                lhsT=lhsT,
                                rhs=ow[:],
                                start=(done == 1),
                                stop=(done == total),
                            )
                    if not skip_compute:
                        out_hook(bb, acc)

            # ---- hop 1: g = A feat ----
            copy_fn = mybir.ActivationFunctionType.Copy

            def hop1_out(bb, acc):
                sl = slice(bb * BLK, (bb + 1) * BLK)
                nc.vector.tensor_copy(out=gT[:, sl], in_=acc[:])
                tp = psum.tile([128, BLK], f32, tag="tp", space="PSUM")
                nc.tensor.matmul(out=tp[:], lhsT=gT[:, sl], rhs=ident_t[:],
                                 start=True, stop=True)
                gn = npool.tile([BLK, D], f16, tag="gn")
                nc.vector.tensor_copy(out=gn[:], in_=tp[:])
                nc.sync.dma_start(out=cc_in[sl, :], in_=gn[:])

            # ---- hop 2: q = A g, fused epilogue ----
            def hop2_out(bb, acc):
                sl = slice(bb * BLK, (bb + 1) * BLK)
                qT_t = npool.tile([128, BLK], f16, tag="qT")
                nc.vector.tensor_copy(out=qT_t[:], in_=acc[:])
                out_ps = psum.tile([128, BLK], f32, tag="outp", space="PSUM")
                nc.tensor.matmul(out=out_ps[:], lhsT=featT[:, sl], rhs=M_t[0][:],
                                 start=True, stop=False)
                nc.tensor.matmul(out=out_ps[:], lhsT=gT[:, sl], rhs=M_t[1][:],
                                 start=False, stop=False)
                nc.tensor.matmul(out=out_ps[:], lhsT=qT_t[:], rhs=M_t[2][:],
                                 start=False, stop=True)
                ob = npool.tile([BLK, D], f16, tag="ob")
                nc.vector.tensor_tensor(out=ob[:], in0=out_ps[:], in1=bias_t[:],
                                        op=mybir.AluOpType.add)
                nc.sync.dma_start(out=out_d[sl, :], in_=ob[:])

            skip_gather = variant == "timing_compute"
            skip_compute = variant == "timing_gather"
            nodep = variant == "timing_nodep"
            noow = variant == "timing_noow"

            if variant == "full":
                # collectives cannot read IO tensors: bounce the local shard
                # through an internal DRAM tile first.
                nc.sync.dma_start(out=featB[:, :], in_=featL_d[:])
                nc.gpsimd.collective_compute(
                    "AllGather",
                    mybir.AluOpType.bypass,
                    ins=[featB.opt()],
                    outs=[featH.opt()],
                    replica_groups=[list(range(NCORE))],
                )
            else:
                # timing variants skip collectives; seed the gather sources
                # so the scheduler sees them written before read.
                nc.sync.dma_start(out=featH[:NPC, :], in_=featL_d[:])
                nc.sync.dma_start(out=cc_out[:NPC, :], in_=featL_d[:])

            def hops_body():
                run_hop(
                    [featH[c * CHUNK : (c + 1) * CHUNK, :] for c in range(NCHUNK)],
                    hop1_out,
                    skip_gather=skip_gather,
                    skip_compute=skip_compute,
                    nodep=nodep,
                    noow=noow,
                )
                if variant == "full":
                    nc.gpsimd.collective_compute(
                        "AllGather",
                        mybir.AluOpType.bypass,
                        ins=[cc_in.opt()],
                        outs=[cc_out.opt()],
                        replica_groups=[list(range(NCORE))],
                    )
                run_hop(
                    [cc_out[c * CHUNK : (c + 1) * CHUNK, :] for c in range(NCHUNK)],
                    hop2_out,
                    skip_gather=skip_gather,
                    skip_compute=skip_compute,
                    nodep=nodep,
                    noow=noow,
                )

            if variant.startswith("timing") and reps > 1:
                with tc.For_i(0, reps, 1):
                    hops_body()
            else:
                hops_body()

    nc.compile()
    return nc


def kernel(feat, W, bias, lambda_max, src, dst):
    in_maps, plan = _prep(feat, W, bias, lambda_max, src, dst)
    nc = _build(plan)
    res = bass_utils.run_bass_kernel_spmd(nc, in_maps, core_ids=list(range(NCORE)))
    # stashed for external benchmarking harnesses (not used by the kernel)
    kernel.last_nc = nc
    kernel.last_in_maps = in_maps
    kernel.last_plan = plan
    out = np.concatenate([res.results[k]["out"] for k in range(NCORE)], axis=0)
    return np.ascontiguousarray(out[: plan["N"]]).astype(np.float32)
